# revision 1
# baseline (speedup 1.0000x reference)
"""LookupConv2d Trainium2 kernel.

Math: out = conv2d(x, W), W[o] = sum_s coeff[o,s] * dictionary[idx[o,s]].
Factorization: W = M @ D where M[o,d] = sum_{s: idx[o,s]=d} coeff[o,s] is a
(512, 100) scatter of the coefficients.  Then
    out = M @ conv2d(x, dictionary)
i.e. a 100-channel conv (23 GFLOP) followed by a 1x1 512x100 mix (5 GFLOP)
instead of a 512-channel conv (118 GFLOP) -- 4.2x fewer FLOPs.

Precision: the TensorE f32r mode streams 1 row/cycle (4x faster than fp32
mode) but rounds operands to 12 significant bits (RNE, measured on HW).
We split x and the dictionary into exact 12-bit halves (xh = top 12 bits,
xl = remainder, both f32r-invariant) and accumulate
    xh*wh + xl*wh + xh*wl
in fp32 PSUM -- full fp32-class accuracy (only xl*wl ~ 2^-24 dropped) at
3 cycles/row instead of fp32 mode's 4.  The small 1x1 mix stays in native
fp32 mode.

Sharding: data-parallel over batch N=16 -> 2 images per core on 8 cores.
dictionary (as [128,100] lhsT tap matrices) and M^T are replicated.
"""

import numpy as np

N_CORES = 8
IMGS_PER_CORE = 2
CIN = 256
COUT = 512
NDICT = 100
H = W = 56
HP = WP = 58  # padded
ROWS_PER_TILE = 8
N_TILES = H // ROWS_PER_TILE  # 7
FREE = ROWS_PER_TILE * W  # 448
S = 3  # lookup sparsity

TRACE = False  # set by test.py to get a profile
_LAST_RESULTS = {}  # test.py reads exec_time_ns from here


def split12(a):
    """Exact split a = hi + lo with <=12 significant bits each (a ~ N(0,1),
    so no denormal/overflow concerns).  Both halves pass through the f32r
    12-bit RNE rounding unchanged."""
    a = np.ascontiguousarray(a, dtype=np.float32)
    hi = (a.view(np.uint32) & np.uint32(0xFFFFF000)).view(np.float32)
    lo = (a - hi).astype(np.float32)
    return hi, lo


def _build_program():
    import concourse.bacc as bacc
    import concourse.mybir as mybir
    import concourse.tile as tile

    f32 = mybir.dt.float32
    f32r = mybir.dt.float32r

    nc = bacc.Bacc("TRN2", target_bir_lowering=False, debug=False)

    xh_d = nc.dram_tensor("xh", (IMGS_PER_CORE, CIN, HP, WP), f32,
                          kind="ExternalInput")
    xl_d = nc.dram_tensor("xl", (IMGS_PER_CORE, CIN, HP, WP), f32,
                          kind="ExternalInput")
    wh_d = nc.dram_tensor("wh", (128, 2 * 9 * NDICT), f32, kind="ExternalInput")
    wl_d = nc.dram_tensor("wl", (128, 2 * 9 * NDICT), f32, kind="ExternalInput")
    mh_d = nc.dram_tensor("mh", (NDICT, COUT), f32, kind="ExternalInput")
    ml_d = nc.dram_tensor("ml", (NDICT, COUT), f32, kind="ExternalInput")
    out_d = nc.dram_tensor("out", (IMGS_PER_CORE, COUT, H, W), f32,
                           kind="ExternalOutput")

    # row chunks of the padded input: first 10 rows, then 6x8 -- tile t only
    # needs chunks 0..t so compute starts after the first chunk lands
    row_chunks = [(0, 10)] + [(10 + 8 * k, 8) for k in range(6)]

    with tile.TileContext(nc) as tc:
        with (
            tc.tile_pool(name="consts", bufs=1) as consts,
            tc.tile_pool(name="xpool", bufs=1) as xpool,
            tc.tile_pool(name="ypool", bufs=3) as ypool,
            tc.tile_pool(name="opool", bufs=8) as opool,
            tc.tile_pool(name="psum_y", bufs=2, space="PSUM") as psum_y_pool,
            tc.tile_pool(name="psum_o", bufs=4, space="PSUM") as psum_o_pool,
        ):
            wh_sb = consts.tile([128, 2 * 9 * NDICT], f32r)
            nc.sync.dma_start(wh_sb[:], wh_d[:].bitcast(f32r))
            wl_sb = consts.tile([128, 2 * 9 * NDICT], f32r)
            nc.sync.dma_start(wl_sb[:], wl_d[:].bitcast(f32r))
            mh_sb = consts.tile([NDICT, COUT], f32r)
            nc.sync.dma_start(mh_sb[:], mh_d[:].bitcast(f32r))
            ml_sb = consts.tile([NDICT, COUT], f32r)
            nc.sync.dma_start(ml_sb[:], ml_d[:].bitcast(f32r))

            # [128 cin-in-block, img, cblk, hp, wp]
            xh_sb = xpool.tile([128, IMGS_PER_CORE, 2, HP, WP], f32r,
                               tag="xh_sb")
            xl_sb = xpool.tile([128, IMGS_PER_CORE, 2, HP, WP], f32r,
                               tag="xl_sb")
            xh_v = xh_d.rearrange("i (b c) h w -> c i b h w", c=128)
            xl_v = xl_d.rearrange("i (b c) h w -> c i b h w", c=128)
            for img in range(IMGS_PER_CORE):
                for r0, nr in row_chunks:
                    for cb in range(2):
                        nc.sync.dma_start(
                            xh_sb[:, img, cb, r0:r0 + nr, :],
                            xh_v[:, img, cb, r0:r0 + nr, :].bitcast(f32r))
                        nc.sync.dma_start(
                            xl_sb[:, img, cb, r0:r0 + nr, :],
                            xl_v[:, img, cb, r0:r0 + nr, :].bitcast(f32r))

            out_v = out_d.rearrange("i (b o) h w -> i b o (h w)", o=128)

            n_mm = 3 * 18

            def emit_conv(img, h0):
                py = psum_y_pool.tile([NDICT, FREE], f32)
                k = 0
                for cb in range(2):
                    for ti in range(3):
                        for tj in range(3):
                            tap = slice((cb * 9 + ti * 3 + tj) * NDICT,
                                        (cb * 9 + ti * 3 + tj + 1) * NDICT)
                            rh = (slice(None), img, cb,
                                  slice(h0 + ti, h0 + ti + ROWS_PER_TILE),
                                  slice(tj, tj + W))
                            for lhsT, rhs in (
                                (wh_sb[:, tap], xh_sb[rh]),
                                (wh_sb[:, tap], xl_sb[rh]),
                                (wl_sb[:, tap], xh_sb[rh]),
                            ):
                                nc.tensor.matmul(
                                    py[:], lhsT, rhs,
                                    start=(k == 0), stop=(k == n_mm - 1))
                                k += 1
                return py

            def emit_mix(py, img, h0):
                # Veltkamp split y = yh + yl into 12-bit halves (pure fp32
                # arithmetic; values are exactly f32r-representable so the
                # matmul's internal rounding is the identity)
                t_sb = ypool.tile([NDICT, FREE], f32, tag="t")
                big = ypool.tile([NDICT, FREE], f32, tag="big")
                yh = ypool.tile([NDICT, FREE], f32r, tag="yh")
                yl = ypool.tile([NDICT, FREE], f32r, tag="yl")
                nc.scalar.mul(t_sb[:], py[:], 4097.0)
                nc.vector.tensor_sub(big[:], t_sb[:], py[:])
                nc.vector.tensor_sub(yh[:], t_sb[:], big[:])
                nc.vector.tensor_sub(yl[:], py[:], yh[:])
                for ob in range(4):
                    obs = slice(ob * 128, (ob + 1) * 128)
                    po = psum_o_pool.tile([128, FREE], f32)
                    nc.tensor.matmul(po[:], mh_sb[:, obs], yh[:],
                                     start=True, stop=False)
                    nc.tensor.matmul(po[:], ml_sb[:, obs], yh[:],
                                     start=False, stop=False)
                    nc.tensor.matmul(po[:], mh_sb[:, obs], yl[:],
                                     start=False, stop=True)
                    o_sb = opool.tile([128, FREE], f32)
                    if ob % 2 == 0:
                        nc.vector.tensor_copy(o_sb[:], po[:])
                    else:
                        nc.scalar.copy(o_sb[:], po[:])
                    nc.sync.dma_start(
                        out_v[img, ob, :, h0 * W:h0 * W + FREE], o_sb[:])

            # software-pipeline by one tile: PE runs tile i's conv while
            # ACT/DVE run tile i-1's Veltkamp split, so the mix matmuls are
            # ready when PE gets to them
            pending = None
            for img in range(IMGS_PER_CORE):
                for t in range(N_TILES):
                    h0 = t * ROWS_PER_TILE
                    py = emit_conv(img, h0)
                    if pending is not None:
                        emit_mix(*pending)
                    pending = (py, img, h0)
            emit_mix(*pending)

    nc.compile()
    return nc


_NC_CACHE = None


def kernel(x, dictionary, lookup_indices, lookup_coefficients):
    global _NC_CACHE
    from concourse import bass_utils

    x = np.asarray(x, dtype=np.float32)
    dictionary = np.asarray(dictionary, dtype=np.float32)
    idx = np.asarray(lookup_indices).astype(np.int64)
    coef = np.asarray(lookup_coefficients, dtype=np.float32)

    # M^T[d, o] = sum_s coeff[o, s] * [idx[o, s] == d]
    mt = np.zeros((NDICT, COUT), np.float32)
    np.add.at(mt, (idx.reshape(-1),
                   np.repeat(np.arange(COUT), S)), coef.reshape(-1))

    # wt[c_in_block, (cblk, ti, tj, d)] = dictionary[d, cblk*128+c, ti, tj]
    wt = np.ascontiguousarray(
        dictionary.reshape(NDICT, 2, 128, 3, 3).transpose(2, 1, 3, 4, 0)
    ).reshape(128, 2 * 9 * NDICT)
    wh, wl = split12(wt)
    mh, ml = split12(mt)

    xp = np.pad(x, ((0, 0), (0, 0), (1, 1), (1, 1)))
    xp = np.ascontiguousarray(
        xp.reshape(N_CORES, IMGS_PER_CORE, CIN, HP, WP))
    xh, xl = split12(xp)

    if _NC_CACHE is None:
        _NC_CACHE = _build_program()
    nc = _NC_CACHE

    in_maps = [{"xh": xh[i], "xl": xl[i], "wh": wh, "wl": wl,
                "mh": mh, "ml": ml} for i in range(N_CORES)]
    try:
        res = bass_utils.run_bass_kernel_spmd(
            nc, in_maps, core_ids=list(range(N_CORES)), trace=TRACE)
    except ModuleNotFoundError:
        # no axon NTFF profile hook in this environment
        res = bass_utils.run_bass_kernel_spmd(
            nc, in_maps, core_ids=list(range(N_CORES)), trace=False)
    _LAST_RESULTS["res"] = res

    out = np.concatenate([r["out"] for r in res.results], axis=0)
    return out.reshape(16, COUT, H, W)



# revision 2
# speedup vs baseline: 2.6442x; 2.6442x over previous
"""LookupConv2d Trainium2 kernel.

Math: out = conv2d(x, W), W[o] = sum_s coeff[o,s] * dictionary[idx[o,s]].
Factorization: W = M @ D where M[o,d] = sum_{s: idx[o,s]=d} coeff[o,s] is a
(512, 100) scatter of the coefficients.  Then
    out = M @ conv2d(x, dictionary)
i.e. a 100-channel conv (23 GFLOP) followed by a 1x1 512x100 mix (5 GFLOP)
instead of a 512-channel conv (118 GFLOP) -- 4.2x fewer FLOPs.

Precision: fp16 operands (11-bit mantissa) with fp32 PSUM accumulation.
TensorE streams fp16 at 1 row/cycle (same rate as bf16/f32r, 4x fp32 mode).
Per-operand rounding is ~2^-12 relative; over the 2304-term conv reduction
the output error stays ~3e-4 relative -- far inside the 2e-2 gate -- while
using a third of the PE cycles of the fp32-class split-matmul scheme and
half the input DMA bytes.

Sharding: data-parallel over batch N=16 -> 2 images per core on 8 cores.
dictionary (as [128, 1800] fp16 tap matrices) and M^T are replicated.
"""

import numpy as np

N_CORES = 8
IMGS_PER_CORE = 2
CIN = 256
COUT = 512
NDICT = 100
H = W = 56
HP = WP = 58  # padded
ROWS_PER_TILE = 8
N_TILES = H // ROWS_PER_TILE  # 7
FREE = ROWS_PER_TILE * W  # 448
S = 3  # lookup sparsity

TRACE = False  # set by test.py to get a profile
_LAST_RESULTS = {}  # test.py reads exec_time_ns from here


def _build_program():
    import concourse.bacc as bacc
    import concourse.mybir as mybir
    import concourse.tile as tile

    f32 = mybir.dt.float32
    f16 = mybir.dt.float16

    nc = bacc.Bacc("TRN2", target_bir_lowering=False, debug=False)

    x_d = nc.dram_tensor("x", (IMGS_PER_CORE, CIN, HP, WP), f16,
                         kind="ExternalInput")
    w_d = nc.dram_tensor("w", (128, 2 * 9 * NDICT), f16, kind="ExternalInput")
    m_d = nc.dram_tensor("m", (NDICT, COUT), f16, kind="ExternalInput")
    out_d = nc.dram_tensor("out", (IMGS_PER_CORE, COUT, H, W), f32,
                           kind="ExternalOutput")

    # row chunks of the padded input: first 10 rows, then 6x8 -- tile t only
    # needs chunks 0..t so compute starts after the first chunk lands
    row_chunks = [(0, 10)] + [(10 + 8 * k, 8) for k in range(6)]

    with tile.TileContext(nc) as tc:
        with (
            tc.tile_pool(name="consts", bufs=1) as consts,
            tc.tile_pool(name="xpool", bufs=1) as xpool,
            tc.tile_pool(name="ypool", bufs=3) as ypool,
            tc.tile_pool(name="opool", bufs=8) as opool,
            tc.tile_pool(name="psum_y", bufs=2, space="PSUM") as psum_y_pool,
            tc.tile_pool(name="psum_o", bufs=4, space="PSUM") as psum_o_pool,
        ):
            w_sb = consts.tile([128, 2 * 9 * NDICT], f16)
            nc.sync.dma_start(w_sb[:], w_d[:])
            m_sb = consts.tile([NDICT, COUT], f16)
            nc.sync.dma_start(m_sb[:], m_d[:])

            # [128 cin-in-block, img, cblk, hp, wp]
            x_sb = xpool.tile([128, IMGS_PER_CORE, 2, HP, WP], f16, tag="x_sb")
            x_v = x_d.rearrange("i (b c) h w -> c i b h w", c=128)
            for img in range(IMGS_PER_CORE):
                for r0, nr in row_chunks:
                    for cb in range(2):
                        nc.sync.dma_start(
                            x_sb[:, img, cb, r0:r0 + nr, :],
                            x_v[:, img, cb, r0:r0 + nr, :])

            out_v = out_d.rearrange("i (b o) h w -> i b o (h w)", o=128)

            n_mm = 18

            def emit_conv(img, h0):
                py = psum_y_pool.tile([NDICT, FREE], f32)
                k = 0
                for cb in range(2):
                    for ti in range(3):
                        for tj in range(3):
                            tap = slice((cb * 9 + ti * 3 + tj) * NDICT,
                                        (cb * 9 + ti * 3 + tj + 1) * NDICT)
                            rh = (slice(None), img, cb,
                                  slice(h0 + ti, h0 + ti + ROWS_PER_TILE),
                                  slice(tj, tj + W))
                            nc.tensor.matmul(
                                py[:], w_sb[:, tap], x_sb[rh],
                                start=(k == 0), stop=(k == n_mm - 1))
                            k += 1
                return py

            def emit_mix(py, img, h0):
                y16 = ypool.tile([NDICT, FREE], f16, tag="y16")
                nc.scalar.copy(y16[:], py[:])
                for ob in range(4):
                    obs = slice(ob * 128, (ob + 1) * 128)
                    po = psum_o_pool.tile([128, FREE], f32)
                    nc.tensor.matmul(po[:], m_sb[:, obs], y16[:],
                                     start=True, stop=True)
                    o_sb = opool.tile([128, FREE], f32)
                    if ob % 2 == 0:
                        nc.vector.tensor_copy(o_sb[:], po[:])
                    else:
                        nc.scalar.copy(o_sb[:], po[:])
                    nc.sync.dma_start(
                        out_v[img, ob, :, h0 * W:h0 * W + FREE], o_sb[:])

            # software-pipeline by one tile: PE runs tile i's conv while
            # ACT/DVE copy tile i-1's PSUM, so the mix matmuls are ready
            # when PE gets to them
            pending = None
            for img in range(IMGS_PER_CORE):
                for t in range(N_TILES):
                    h0 = t * ROWS_PER_TILE
                    py = emit_conv(img, h0)
                    if pending is not None:
                        emit_mix(*pending)
                    pending = (py, img, h0)
            emit_mix(*pending)

    nc.compile()
    return nc


_NC_CACHE = None


def kernel(x, dictionary, lookup_indices, lookup_coefficients):
    global _NC_CACHE
    from concourse import bass_utils

    x = np.asarray(x, dtype=np.float32)
    dictionary = np.asarray(dictionary, dtype=np.float32)
    idx = np.asarray(lookup_indices).astype(np.int64)
    coef = np.asarray(lookup_coefficients, dtype=np.float32)

    # M^T[d, o] = sum_s coeff[o, s] * [idx[o, s] == d]
    mt = np.zeros((NDICT, COUT), np.float32)
    np.add.at(mt, (idx.reshape(-1),
                   np.repeat(np.arange(COUT), S)), coef.reshape(-1))

    # wt[c_in_block, (cblk, ti, tj, d)] = dictionary[d, cblk*128+c, ti, tj]
    wt = np.ascontiguousarray(
        dictionary.reshape(NDICT, 2, 128, 3, 3).transpose(2, 1, 3, 4, 0)
    ).reshape(128, 2 * 9 * NDICT)

    xp = np.pad(x, ((0, 0), (0, 0), (1, 1), (1, 1)))
    xp = np.ascontiguousarray(
        xp.reshape(N_CORES, IMGS_PER_CORE, CIN, HP, WP)).astype(np.float16)
    wt16 = wt.astype(np.float16)
    mt16 = mt.astype(np.float16)

    if _NC_CACHE is None:
        _NC_CACHE = _build_program()
    nc = _NC_CACHE

    in_maps = [{"x": xp[i], "w": wt16, "m": mt16} for i in range(N_CORES)]
    try:
        res = bass_utils.run_bass_kernel_spmd(
            nc, in_maps, core_ids=list(range(N_CORES)), trace=TRACE)
    except ModuleNotFoundError:
        # no axon NTFF profile hook in this environment
        res = bass_utils.run_bass_kernel_spmd(
            nc, in_maps, core_ids=list(range(N_CORES)), trace=False)
    _LAST_RESULTS["res"] = res

    out = np.concatenate([r["out"] for r in res.results], axis=0)
    return out.reshape(16, COUT, H, W)


# revision 6
# speedup vs baseline: 2.6835x; 1.0149x over previous
"""LookupConv2d Trainium2 kernel.

Math: out = conv2d(x, W), W[o] = sum_s coeff[o,s] * dictionary[idx[o,s]].
Factorization: W = M @ D where M[o,d] = sum_{s: idx[o,s]=d} coeff[o,s] is a
(512, 100) scatter of the coefficients.  Then
    out = M @ conv2d(x, dictionary)
i.e. a 100-channel conv (23 GFLOP) followed by a 1x1 512x100 mix (5 GFLOP)
instead of a 512-channel conv (118 GFLOP) -- 4.2x fewer FLOPs.

Precision: fp16 operands (11-bit mantissa) with fp32 PSUM accumulation.
TensorE streams fp16 at 1 row/cycle (same rate as bf16/f32r, 4x fp32 mode).
Per-operand rounding is ~2^-12 relative; over the 2304-term conv reduction
the output error stays ~4e-4 relative -- far inside the 2e-2 gate -- while
using a third of the PE cycles of the fp32-class split-matmul scheme and
half the input DMA bytes.

Schedule: row-tiles of 9,9,9,9,9,8,3 rows per image (the 3-row tail keeps
the end-of-kernel mix+copy+DMA chain short).  PE is pre-warmed with dummy
matmuls on a zeroed scratch tile so the clock is fully ramped when the
first input chunk lands.  Output is staged in SBUF and written with a few
large DMAs per image instead of one per tile.

Sharding: data-parallel over batch N=16 -> 2 images per core on 8 cores.
dictionary (as [128, 1800] fp16 tap matrices) and M^T are replicated.
"""

import numpy as np

N_CORES = 8
IMGS_PER_CORE = 2
CIN = 256
COUT = 512
NDICT = 100
H = W = 56
HP = WP = 58  # padded
S = 3  # lookup sparsity
HW = H * W  # 3136

# (h0, rows) conv tiles per image; free dim = rows*W <= 504 (PSUM bank cap)
TILES = [(0, 9), (9, 9), (18, 9), (27, 9), (36, 9), (45, 8), (53, 3)]
# padded-input row chunks; tile t only needs chunks 0..ceil
ROW_CHUNKS = [(0, 11), (11, 9), (20, 9), (29, 9), (38, 9), (47, 11)]
MAX_FREE = 9 * W  # 504

TRACE = False  # set by test.py to get a profile
_LAST_RESULTS = {}  # test.py reads exec_time_ns from here


def _build_program():
    import concourse.bacc as bacc
    import concourse.mybir as mybir
    import concourse.tile as tile

    f32 = mybir.dt.float32
    f16 = mybir.dt.float16

    nc = bacc.Bacc("TRN2", target_bir_lowering=False, debug=False)

    x_d = nc.dram_tensor("x", (IMGS_PER_CORE, CIN, HP, WP), f16,
                         kind="ExternalInput")
    w_d = nc.dram_tensor("w", (128, 2 * 9 * NDICT), f16, kind="ExternalInput")
    m_d = nc.dram_tensor("m", (NDICT, COUT), f16, kind="ExternalInput")
    out_d = nc.dram_tensor("out", (IMGS_PER_CORE, COUT, H, W), f32,
                           kind="ExternalOutput")

    with tile.TileContext(nc) as tc:
        with (
            tc.tile_pool(name="warm", bufs=1) as warm,
            tc.tile_pool(name="consts", bufs=1) as consts,
            tc.tile_pool(name="xpool", bufs=1) as xpool,
            tc.tile_pool(name="ypool", bufs=3) as ypool,
            tc.tile_pool(name="opool", bufs=2) as opool,
            tc.tile_pool(name="psum_w", bufs=1, space="PSUM") as psum_w_pool,
            tc.tile_pool(name="psum_y", bufs=2, space="PSUM") as psum_y_pool,
            tc.tile_pool(name="psum_o", bufs=4, space="PSUM") as psum_o_pool,
        ):
            # --- PE pre-warm: dummy matmuls on zeroed scratch keep the PE
            # clock ramping while the first weight/input DMAs are in flight
            scratch = warm.tile([128, 128], f16)
            nc.vector.memset(scratch[:], 0.0)
            pwarm = psum_w_pool.tile([128, 128], f32)
            for _ in range(22):
                nc.tensor.matmul(pwarm[:], scratch[:], scratch[:],
                                 start=True, stop=True)

            # --- constants + input, ordered so the first conv tile's
            # dependencies (w cb0 taps, x img0 chunk0) land first
            w_sb = consts.tile([128, 2 * 9 * NDICT], f16)
            m_sb = consts.tile([NDICT, COUT], f16)
            x_sb = xpool.tile([128, IMGS_PER_CORE, 2, HP, WP], f16, tag="x_sb")
            x_v = x_d.rearrange("i (b c) h w -> c i b h w", c=128)

            def dma_x(img, cb, chunk):
                r0, nr = ROW_CHUNKS[chunk]
                nc.sync.dma_start(x_sb[:, img, cb, r0:r0 + nr, :],
                                  x_v[:, img, cb, r0:r0 + nr, :])

            nc.sync.dma_start(w_sb[:, :900], w_d[:, :900])
            dma_x(0, 0, 0)
            nc.sync.dma_start(w_sb[:, 900:], w_d[:, 900:])
            dma_x(0, 1, 0)
            nc.sync.dma_start(m_sb[:], m_d[:])
            for img in range(IMGS_PER_CORE):
                for chunk in range(1 if img == 0 else 0, len(ROW_CHUNKS)):
                    for cb in range(2):
                        dma_x(img, cb, chunk)

            out_v = out_d.rearrange("i (b o) h w -> i b o (h w)", o=128)
            # same tensor viewed [img, o, b, hw] so a [128, 4, cols] SBUF
            # tile maps element-for-element in the tail DMA
            out_vt = out_d.rearrange("i (b o) h w -> i o b (h w)", o=128)

            def emit_conv(img, ti_idx):
                h0, rows = TILES[ti_idx]
                fd = rows * W
                py = psum_y_pool.tile([NDICT, MAX_FREE], f32, tag="py")
                k = 0
                for cb in range(2):
                    for ti in range(3):
                        for tj in range(3):
                            tap = slice((cb * 9 + ti * 3 + tj) * NDICT,
                                        (cb * 9 + ti * 3 + tj + 1) * NDICT)
                            rh = (slice(None), img, cb,
                                  slice(h0 + ti, h0 + ti + rows),
                                  slice(tj, tj + W))
                            nc.tensor.matmul(
                                py[:, :fd], w_sb[:, tap], x_sb[rh],
                                start=(k == 0), stop=(k == 17))
                            k += 1
                return py

            o_accs = {}

            def emit_mix(py, img, ti_idx):
                h0, rows = TILES[ti_idx]
                fd = rows * W
                c0 = h0 * W
                if ti_idx == 0:
                    o_accs[img] = opool.tile([128, 4, HW], f32, tag="oacc",
                                             name=f"oacc{img}")
                o_acc = o_accs[img]
                y16 = ypool.tile([NDICT, MAX_FREE], f16, tag="y16")
                nc.scalar.copy(y16[:, :fd], py[:, :fd])
                for ob in range(4):
                    obs = slice(ob * 128, (ob + 1) * 128)
                    po = psum_o_pool.tile([128, MAX_FREE], f32, tag="po")
                    nc.tensor.matmul(po[:, :fd], m_sb[:, obs], y16[:, :fd],
                                     start=True, stop=True)
                    dst = o_acc[:, ob, c0:c0 + fd]
                    if ob % 2 == 0:
                        nc.vector.tensor_copy(dst, po[:, :fd])
                    else:
                        nc.scalar.copy(dst, po[:, :fd])
                    # stream finished column ranges out as they complete:
                    # tiles 0-2 cover cols 0:1512, tiles 3-5 cover 1512:2968
                    if ti_idx == 2:
                        nc.sync.dma_start(out_v[img, ob, :, 0:1512],
                                          o_acc[:, ob, 0:1512])
                    elif ti_idx == 5:
                        nc.sync.dma_start(out_v[img, ob, :, 1512:2968],
                                          o_acc[:, ob, 1512:2968])
                if ti_idx == 6:
                    # tail: all 4 ob blocks' last 3 rows in one DMA
                    nc.sync.dma_start(out_vt[img, :, :, 2968:HW],
                                      o_acc[:, :, 2968:HW])

            # software-pipeline by one tile: PE runs tile i's conv while
            # ACT/DVE copy tile i-1's PSUM, so the mix matmuls are ready
            # when PE gets to them
            pending = None
            for img in range(IMGS_PER_CORE):
                for t in range(len(TILES)):
                    py = emit_conv(img, t)
                    if pending is not None:
                        emit_mix(*pending)
                    pending = (py, img, t)
            emit_mix(*pending)

    nc.compile()
    return nc


_NC_CACHE = None


def kernel(x, dictionary, lookup_indices, lookup_coefficients):
    global _NC_CACHE
    from concourse import bass_utils

    x = np.asarray(x, dtype=np.float32)
    dictionary = np.asarray(dictionary, dtype=np.float32)
    idx = np.asarray(lookup_indices).astype(np.int64)
    coef = np.asarray(lookup_coefficients, dtype=np.float32)

    # M^T[d, o] = sum_s coeff[o, s] * [idx[o, s] == d]
    mt = np.zeros((NDICT, COUT), np.float32)
    np.add.at(mt, (idx.reshape(-1),
                   np.repeat(np.arange(COUT), S)), coef.reshape(-1))

    # wt[c_in_block, (cblk, ti, tj, d)] = dictionary[d, cblk*128+c, ti, tj]
    wt = np.ascontiguousarray(
        dictionary.reshape(NDICT, 2, 128, 3, 3).transpose(2, 1, 3, 4, 0)
    ).reshape(128, 2 * 9 * NDICT)

    xp = np.pad(x, ((0, 0), (0, 0), (1, 1), (1, 1)))
    xp = np.ascontiguousarray(
        xp.reshape(N_CORES, IMGS_PER_CORE, CIN, HP, WP)).astype(np.float16)
    wt16 = wt.astype(np.float16)
    mt16 = mt.astype(np.float16)

    if _NC_CACHE is None:
        _NC_CACHE = _build_program()
    nc = _NC_CACHE

    in_maps = [{"x": xp[i], "w": wt16, "m": mt16} for i in range(N_CORES)]
    try:
        res = bass_utils.run_bass_kernel_spmd(
            nc, in_maps, core_ids=list(range(N_CORES)), trace=TRACE)
    except ModuleNotFoundError:
        # no axon NTFF profile hook in this environment
        res = bass_utils.run_bass_kernel_spmd(
            nc, in_maps, core_ids=list(range(N_CORES)), trace=False)
    _LAST_RESULTS["res"] = res

    out = np.concatenate([r["out"] for r in res.results], axis=0)
    return out.reshape(16, COUT, H, W)


# revision 8
# speedup vs baseline: 2.8166x; 1.0496x over previous
"""LookupConv2d Trainium2 kernel.

Math: out = conv2d(x, W), W[o] = sum_s coeff[o,s] * dictionary[idx[o,s]].
Factorization: W = M @ D where M[o,d] = sum_{s: idx[o,s]=d} coeff[o,s] is a
(512, 100) scatter of the coefficients.  Then
    out = M @ conv2d(x, dictionary)
i.e. a 100-channel conv (23 GFLOP) followed by a 1x1 512x100 mix (5 GFLOP)
instead of a 512-channel conv (118 GFLOP) -- 4.2x fewer FLOPs.

Precision: fp16 operands (11-bit mantissa) with fp32 PSUM accumulation.
TensorE streams fp16 at 1 row/cycle (same rate as bf16/f32r, 4x fp32 mode).
Per-operand rounding is ~2^-12 relative; over the 2304-term conv reduction
the output error stays ~4e-4 relative -- far inside the 2e-2 gate -- while
using a third of the PE cycles of the fp32-class split-matmul scheme and
half the input DMA bytes.

Schedule: row-tiles of 9,9,9,9,9,8,3 rows per image (the 3-row tail keeps
the end-of-kernel mix+copy+DMA chain short).  PE is pre-warmed with dummy
matmuls on a zeroed scratch tile so the clock is fully ramped when the
first input chunk lands.  Output is staged in SBUF and written with a few
large DMAs per image instead of one per tile.

Sharding: data-parallel over batch N=16 -> 2 images per core on 8 cores.
dictionary (as [128, 1800] fp16 tap matrices) and M^T are replicated.
"""

import numpy as np

N_CORES = 8
IMGS_PER_CORE = 2
CIN = 256
COUT = 512
NDICT = 100
H = W = 56
HP = WP = 58  # padded
S = 3  # lookup sparsity
HW = H * W  # 3136

# (h0, rows) conv tiles per image; free dim = rows*W <= 504 (PSUM bank cap)
TILES = [(0, 9), (9, 9), (18, 9), (27, 9), (36, 9), (45, 8), (53, 3)]
# padded-input row chunks; tile t only needs chunks 0..ceil
ROW_CHUNKS = [(0, 11), (11, 9), (20, 9), (29, 9), (38, 9), (47, 11)]
MAX_FREE = 9 * W  # 504

TRACE = False  # set by test.py to get a profile
_LAST_RESULTS = {}  # test.py reads exec_time_ns from here


def _build_program():
    import concourse.bacc as bacc
    import concourse.mybir as mybir
    import concourse.tile as tile

    f32 = mybir.dt.float32
    f16 = mybir.dt.float16

    nc = bacc.Bacc("TRN2", target_bir_lowering=False, debug=False)

    x_d = nc.dram_tensor("x", (IMGS_PER_CORE, CIN, HP, WP), f16,
                         kind="ExternalInput")
    w_d = nc.dram_tensor("w", (128, 2 * 9 * NDICT), f16, kind="ExternalInput")
    m_d = nc.dram_tensor("m", (NDICT, COUT), f16, kind="ExternalInput")
    out_d = nc.dram_tensor("out", (IMGS_PER_CORE, COUT, H, W), f32,
                           kind="ExternalOutput")

    with tile.TileContext(nc) as tc:
        with (
            tc.tile_pool(name="warm", bufs=1) as warm,
            tc.tile_pool(name="consts", bufs=1) as consts,
            tc.tile_pool(name="xpool", bufs=1) as xpool,
            tc.tile_pool(name="ypool", bufs=3) as ypool,
            tc.tile_pool(name="opool", bufs=2) as opool,
            tc.tile_pool(name="psum_w", bufs=1, space="PSUM") as psum_w_pool,
            tc.tile_pool(name="psum_y", bufs=2, space="PSUM") as psum_y_pool,
            tc.tile_pool(name="psum_o", bufs=4, space="PSUM") as psum_o_pool,
        ):
            # --- PE pre-warm: dummy matmuls on zeroed scratch keep the PE
            # clock ramping while the first weight/input DMAs are in flight
            scratch = warm.tile([128, 128], f16)
            nc.vector.memset(scratch[:], 0.0)
            pwarm = psum_w_pool.tile([128, 128], f32)
            for _ in range(22):
                nc.tensor.matmul(pwarm[:], scratch[:], scratch[:],
                                 start=True, stop=True)

            # --- constants + input, ordered so the first conv tile's
            # dependencies (w cb0 taps, x img0 chunk0) land first
            w_sb = consts.tile([128, 2 * 9 * NDICT], f16)
            m_sb = consts.tile([NDICT, COUT], f16)
            x_sb = xpool.tile([128, IMGS_PER_CORE, 2, HP, WP], f16, tag="x_sb")
            x_v = x_d.rearrange("i (b c) h w -> c i b h w", c=128)

            def dma_x(img, cb, chunk):
                r0, nr = ROW_CHUNKS[chunk]
                nc.sync.dma_start(x_sb[:, img, cb, r0:r0 + nr, :],
                                  x_v[:, img, cb, r0:r0 + nr, :])

            nc.sync.dma_start(w_sb[:, :100], w_d[:, :100])
            dma_x(0, 0, 0)
            nc.sync.dma_start(w_sb[:, 100:900], w_d[:, 100:900])
            dma_x(0, 1, 0)
            nc.sync.dma_start(w_sb[:, 900:], w_d[:, 900:])
            nc.sync.dma_start(m_sb[:], m_d[:])
            for img in range(IMGS_PER_CORE):
                for chunk in range(1 if img == 0 else 0, len(ROW_CHUNKS)):
                    for cb in range(2):
                        dma_x(img, cb, chunk)

            out_v = out_d.rearrange("i (b o) h w -> i b o (h w)", o=128)
            # same tensor viewed [img, o, b, hw] so a [128, 4, cols] SBUF
            # tile maps element-for-element in the tail DMA
            out_vt = out_d.rearrange("i (b o) h w -> i o b (h w)", o=128)

            def emit_conv(img, ti_idx):
                h0, rows = TILES[ti_idx]
                fd = rows * W
                py = psum_y_pool.tile([NDICT, MAX_FREE], f32, tag="py")
                k = 0
                for cb in range(2):
                    for ti in range(3):
                        for tj in range(3):
                            tap = slice((cb * 9 + ti * 3 + tj) * NDICT,
                                        (cb * 9 + ti * 3 + tj + 1) * NDICT)
                            rh = (slice(None), img, cb,
                                  slice(h0 + ti, h0 + ti + rows),
                                  slice(tj, tj + W))
                            nc.tensor.matmul(
                                py[:, :fd], w_sb[:, tap], x_sb[rh],
                                start=(k == 0), stop=(k == 17))
                            k += 1
                return py

            o_accs = {}

            def emit_mix(py, img, ti_idx):
                h0, rows = TILES[ti_idx]
                fd = rows * W
                c0 = h0 * W
                if ti_idx == 0:
                    o_accs[img] = opool.tile([128, 4, HW], f32, tag="oacc",
                                             name=f"oacc{img}")
                o_acc = o_accs[img]
                y16 = ypool.tile([NDICT, MAX_FREE], f16, tag="y16")
                nc.scalar.copy(y16[:, :fd], py[:, :fd])
                for ob in range(4):
                    obs = slice(ob * 128, (ob + 1) * 128)
                    po = psum_o_pool.tile([128, MAX_FREE], f32, tag="po")
                    nc.tensor.matmul(po[:, :fd], m_sb[:, obs], y16[:, :fd],
                                     start=True, stop=True)
                    dst = o_acc[:, ob, c0:c0 + fd]
                    if ob % 2 == 0:
                        nc.vector.tensor_copy(dst, po[:, :fd])
                    else:
                        nc.scalar.copy(dst, po[:, :fd])
                # one combined-ob DMA per tile keeps the output stream
                # spread across the whole run (no end-of-kernel backlog)
                nc.sync.dma_start(out_vt[img, :, :, c0:c0 + fd],
                                  o_acc[:, :, c0:c0 + fd])

            # software-pipeline by one tile: PE runs tile i's conv while
            # ACT/DVE copy tile i-1's PSUM, so the mix matmuls are ready
            # when PE gets to them
            pending = None
            for img in range(IMGS_PER_CORE):
                for t in range(len(TILES)):
                    py = emit_conv(img, t)
                    if pending is not None:
                        emit_mix(*pending)
                    pending = (py, img, t)
            emit_mix(*pending)

    nc.compile()
    return nc


_NC_CACHE = None


def kernel(x, dictionary, lookup_indices, lookup_coefficients):
    global _NC_CACHE
    from concourse import bass_utils

    x = np.asarray(x, dtype=np.float32)
    dictionary = np.asarray(dictionary, dtype=np.float32)
    idx = np.asarray(lookup_indices).astype(np.int64)
    coef = np.asarray(lookup_coefficients, dtype=np.float32)

    # M^T[d, o] = sum_s coeff[o, s] * [idx[o, s] == d]
    mt = np.zeros((NDICT, COUT), np.float32)
    np.add.at(mt, (idx.reshape(-1),
                   np.repeat(np.arange(COUT), S)), coef.reshape(-1))

    # wt[c_in_block, (cblk, ti, tj, d)] = dictionary[d, cblk*128+c, ti, tj]
    wt = np.ascontiguousarray(
        dictionary.reshape(NDICT, 2, 128, 3, 3).transpose(2, 1, 3, 4, 0)
    ).reshape(128, 2 * 9 * NDICT)

    xp = np.pad(x, ((0, 0), (0, 0), (1, 1), (1, 1)))
    xp = np.ascontiguousarray(
        xp.reshape(N_CORES, IMGS_PER_CORE, CIN, HP, WP)).astype(np.float16)
    wt16 = wt.astype(np.float16)
    mt16 = mt.astype(np.float16)

    if _NC_CACHE is None:
        _NC_CACHE = _build_program()
    nc = _NC_CACHE

    in_maps = [{"x": xp[i], "w": wt16, "m": mt16} for i in range(N_CORES)]
    try:
        res = bass_utils.run_bass_kernel_spmd(
            nc, in_maps, core_ids=list(range(N_CORES)), trace=TRACE)
    except ModuleNotFoundError:
        # no axon NTFF profile hook in this environment
        res = bass_utils.run_bass_kernel_spmd(
            nc, in_maps, core_ids=list(range(N_CORES)), trace=False)
    _LAST_RESULTS["res"] = res

    out = np.concatenate([r["out"] for r in res.results], axis=0)
    return out.reshape(16, COUT, H, W)


# revision 10
# speedup vs baseline: 2.9102x; 1.0332x over previous
"""LookupConv2d Trainium2 kernel.

Math: out = conv2d(x, W), W[o] = sum_s coeff[o,s] * dictionary[idx[o,s]].
Factorization: W = M @ D where M[o,d] = sum_{s: idx[o,s]=d} coeff[o,s] is a
(512, 100) scatter of the coefficients.  Then
    out = M @ conv2d(x, dictionary)
i.e. a 100-channel conv (23 GFLOP) followed by a 1x1 512x100 mix (5 GFLOP)
instead of a 512-channel conv (118 GFLOP) -- 4.2x fewer FLOPs.

Precision: fp16 operands (11-bit mantissa) with fp32 PSUM accumulation.
TensorE streams fp16 at 1 row/cycle (same rate as bf16/f32r, 4x fp32 mode).
Per-operand rounding is ~2^-12 relative; over the 2304-term conv reduction
the output error stays ~4e-4 relative -- far inside the 2e-2 gate -- while
using a third of the PE cycles of the fp32-class split-matmul scheme and
half the input DMA bytes.

Schedule: row-tiles of 9,9,9,9,9,8,3 rows per image (the 3-row tail keeps
the end-of-kernel mix+copy+DMA chain short).  PE is pre-warmed with dummy
matmuls on a zeroed scratch tile so the clock is fully ramped when the
first input chunk lands.  Output is staged in SBUF and written with a few
large DMAs per image instead of one per tile.

Sharding: data-parallel over batch N=16 -> 2 images per core on 8 cores.
dictionary (as [128, 1800] fp16 tap matrices) and M^T are replicated.
"""

import numpy as np

N_CORES = 8
IMGS_PER_CORE = 2
CIN = 256
COUT = 512
NDICT = 100
H = W = 56
HP = WP = 58  # padded
S = 3  # lookup sparsity
HW = H * W  # 3136

# (h0, rows) conv tiles per image; free dim = rows*W <= 504 (PSUM bank cap)
TILES = [(0, 9), (9, 9), (18, 9), (27, 9), (36, 9), (45, 8), (53, 3)]
# padded-input row chunks; tile t only needs chunks 0..ceil
ROW_CHUNKS = [(0, 11), (11, 9), (20, 9), (29, 9), (38, 9), (47, 11)]
MAX_FREE = 9 * W  # 504

TRACE = False  # set by test.py to get a profile
_LAST_RESULTS = {}  # test.py reads exec_time_ns from here


def _build_program():
    import concourse.bacc as bacc
    import concourse.mybir as mybir
    import concourse.tile as tile

    f32 = mybir.dt.float32
    f16 = mybir.dt.float16

    nc = bacc.Bacc("TRN2", target_bir_lowering=False, debug=False)

    x_d = nc.dram_tensor("x", (IMGS_PER_CORE, CIN, HP, WP), f16,
                         kind="ExternalInput")
    w_d = nc.dram_tensor("w", (128, 2 * 9 * NDICT), f16, kind="ExternalInput")
    m_d = nc.dram_tensor("m", (NDICT, COUT), f16, kind="ExternalInput")
    out_d = nc.dram_tensor("out", (IMGS_PER_CORE, COUT, H, W), f16,
                           kind="ExternalOutput")

    with tile.TileContext(nc) as tc:
        with (
            tc.tile_pool(name="warm", bufs=1) as warm,
            tc.tile_pool(name="consts", bufs=1) as consts,
            tc.tile_pool(name="xpool", bufs=1) as xpool,
            tc.tile_pool(name="ypool", bufs=3) as ypool,
            tc.tile_pool(name="opool", bufs=2) as opool,
            tc.tile_pool(name="psum_w", bufs=1, space="PSUM") as psum_w_pool,
            tc.tile_pool(name="psum_y", bufs=2, space="PSUM") as psum_y_pool,
            tc.tile_pool(name="psum_o", bufs=4, space="PSUM") as psum_o_pool,
        ):
            # --- PE pre-warm: dummy matmuls on zeroed scratch keep the PE
            # clock ramping while the first weight/input DMAs are in flight
            scratch = warm.tile([128, 128], f16)
            nc.vector.memset(scratch[:], 0.0)
            pwarm = psum_w_pool.tile([128, 128], f32)
            for _ in range(22):
                nc.tensor.matmul(pwarm[:], scratch[:], scratch[:],
                                 start=True, stop=True)

            # --- constants + input, ordered so the first conv tile's
            # dependencies (w cb0 taps, x img0 chunk0) land first
            w_sb = consts.tile([128, 2 * 9 * NDICT], f16)
            m_sb = consts.tile([NDICT, COUT], f16)
            x_sb = xpool.tile([128, IMGS_PER_CORE, 2, HP, WP], f16, tag="x_sb")
            x_v = x_d.rearrange("i (b c) h w -> c i b h w", c=128)

            def dma_x(img, cb, chunk):
                r0, nr = ROW_CHUNKS[chunk]
                nc.sync.dma_start(x_sb[:, img, cb, r0:r0 + nr, :],
                                  x_v[:, img, cb, r0:r0 + nr, :])

            nc.sync.dma_start(w_sb[:, :100], w_d[:, :100])
            dma_x(0, 0, 0)
            nc.sync.dma_start(w_sb[:, 100:900], w_d[:, 100:900])
            nc.sync.dma_start(w_sb[:, 900:], w_d[:, 900:])
            dma_x(0, 1, 0)
            nc.sync.dma_start(m_sb[:], m_d[:])
            for img in range(IMGS_PER_CORE):
                for chunk in range(1 if img == 0 else 0, len(ROW_CHUNKS)):
                    for cb in range(2):
                        dma_x(img, cb, chunk)

            out_v = out_d.rearrange("i (b o) h w -> i b o (h w)", o=128)
            # same tensor viewed [img, o, b, hw] so a [128, 4, cols] SBUF
            # tile maps element-for-element in the tail DMA
            out_vt = out_d.rearrange("i (b o) h w -> i o b (h w)", o=128)

            def emit_conv(img, ti_idx):
                h0, rows = TILES[ti_idx]
                fd = rows * W
                py = psum_y_pool.tile([NDICT, MAX_FREE], f32, tag="py")
                k = 0
                for cb in range(2):
                    for ti in range(3):
                        for tj in range(3):
                            tap = slice((cb * 9 + ti * 3 + tj) * NDICT,
                                        (cb * 9 + ti * 3 + tj + 1) * NDICT)
                            rh = (slice(None), img, cb,
                                  slice(h0 + ti, h0 + ti + rows),
                                  slice(tj, tj + W))
                            nc.tensor.matmul(
                                py[:, :fd], w_sb[:, tap], x_sb[rh],
                                start=(k == 0), stop=(k == 17))
                            k += 1
                return py

            o_accs = {}

            def emit_mix(py, img, ti_idx):
                h0, rows = TILES[ti_idx]
                fd = rows * W
                c0 = h0 * W
                if ti_idx == 0:
                    o_accs[img] = opool.tile([128, 4, HW], f16, tag="oacc",
                                             name=f"oacc{img}")
                o_acc = o_accs[img]
                y16 = ypool.tile([NDICT, MAX_FREE], f16, tag="y16")
                nc.scalar.copy(y16[:, :fd], py[:, :fd])
                for ob in range(4):
                    obs = slice(ob * 128, (ob + 1) * 128)
                    po = psum_o_pool.tile([128, MAX_FREE], f32, tag="po")
                    nc.tensor.matmul(po[:, :fd], m_sb[:, obs], y16[:, :fd],
                                     start=True, stop=True)
                    dst = o_acc[:, ob, c0:c0 + fd]
                    if ob % 2 == 0:
                        nc.vector.tensor_copy(dst, po[:, :fd])
                    else:
                        nc.scalar.copy(dst, po[:, :fd])
                # one combined-ob DMA per tile keeps the output stream
                # spread across the whole run (no end-of-kernel backlog)
                nc.sync.dma_start(out_vt[img, :, :, c0:c0 + fd],
                                  o_acc[:, :, c0:c0 + fd])

            # software-pipeline by one tile: PE runs tile i's conv while
            # ACT/DVE copy tile i-1's PSUM, so the mix matmuls are ready
            # when PE gets to them
            pending = None
            for img in range(IMGS_PER_CORE):
                for t in range(len(TILES)):
                    py = emit_conv(img, t)
                    if pending is not None:
                        emit_mix(*pending)
                    pending = (py, img, t)
            emit_mix(*pending)

    nc.compile()
    return nc


_NC_CACHE = None


def kernel(x, dictionary, lookup_indices, lookup_coefficients):
    global _NC_CACHE
    from concourse import bass_utils

    x = np.asarray(x, dtype=np.float32)
    dictionary = np.asarray(dictionary, dtype=np.float32)
    idx = np.asarray(lookup_indices).astype(np.int64)
    coef = np.asarray(lookup_coefficients, dtype=np.float32)

    # M^T[d, o] = sum_s coeff[o, s] * [idx[o, s] == d]
    mt = np.zeros((NDICT, COUT), np.float32)
    np.add.at(mt, (idx.reshape(-1),
                   np.repeat(np.arange(COUT), S)), coef.reshape(-1))

    # wt[c_in_block, (cblk, ti, tj, d)] = dictionary[d, cblk*128+c, ti, tj]
    wt = np.ascontiguousarray(
        dictionary.reshape(NDICT, 2, 128, 3, 3).transpose(2, 1, 3, 4, 0)
    ).reshape(128, 2 * 9 * NDICT)

    xp = np.pad(x, ((0, 0), (0, 0), (1, 1), (1, 1)))
    xp = np.ascontiguousarray(
        xp.reshape(N_CORES, IMGS_PER_CORE, CIN, HP, WP)).astype(np.float16)
    wt16 = wt.astype(np.float16)
    mt16 = mt.astype(np.float16)

    if _NC_CACHE is None:
        _NC_CACHE = _build_program()
    nc = _NC_CACHE

    in_maps = [{"x": xp[i], "w": wt16, "m": mt16} for i in range(N_CORES)]
    try:
        res = bass_utils.run_bass_kernel_spmd(
            nc, in_maps, core_ids=list(range(N_CORES)), trace=TRACE)
    except ModuleNotFoundError:
        # no axon NTFF profile hook in this environment
        res = bass_utils.run_bass_kernel_spmd(
            nc, in_maps, core_ids=list(range(N_CORES)), trace=False)
    _LAST_RESULTS["res"] = res

    out = np.concatenate([r["out"] for r in res.results], axis=0)
    return out.reshape(16, COUT, H, W).astype(np.float32)


# revision 13
# speedup vs baseline: 2.9527x; 1.0146x over previous
"""LookupConv2d Trainium2 kernel.

Math: out = conv2d(x, W), W[o] = sum_s coeff[o,s] * dictionary[idx[o,s]].
Factorization: W = M @ D where M[o,d] = sum_{s: idx[o,s]=d} coeff[o,s] is a
(512, 100) scatter of the coefficients.  Then
    out = M @ conv2d(x, dictionary)
i.e. a 100-channel conv (23 GFLOP) followed by a 1x1 512x100 mix (5 GFLOP)
instead of a 512-channel conv (118 GFLOP) -- 4.2x fewer FLOPs.

Precision: fp16 operands (11-bit mantissa) with fp32 PSUM accumulation.
TensorE streams fp16 at 1 row/cycle (same rate as bf16/f32r, 4x fp32 mode).
Per-operand rounding is ~2^-12 relative; over the 2304-term conv reduction
the output error stays ~4e-4 relative -- far inside the 2e-2 gate -- while
using a third of the PE cycles of the fp32-class split-matmul scheme and
half the input DMA bytes.

Schedule: row-tiles of 9,9,9,9,9,8,3 rows per image (the 3-row tail keeps
the end-of-kernel mix+copy+DMA chain short).  PE is pre-warmed with dummy
matmuls on a zeroed scratch tile so the clock is fully ramped when the
first input chunk lands.  Output is staged in SBUF and written with a few
large DMAs per image instead of one per tile.

Sharding: data-parallel over batch N=16 -> 2 images per core on 8 cores.
dictionary (as [128, 1800] fp16 tap matrices) and M^T are replicated.
"""

import numpy as np

N_CORES = 8
IMGS_PER_CORE = 2
CIN = 256
COUT = 512
NDICT = 100
H = W = 56
HP = WP = 58  # padded
S = 3  # lookup sparsity
HW = H * W  # 3136

# (h0, rows) conv tiles per image; free dim = rows*W <= 504 (PSUM bank cap)
TILES = [(0, 9), (9, 9), (18, 9), (27, 9), (36, 9), (45, 8), (53, 3)]
# padded-input row chunks; tile t only needs chunks 0..ceil
ROW_CHUNKS = [(0, 11), (11, 9), (20, 9), (29, 9), (38, 9), (47, 11)]
MAX_FREE = 9 * W  # 504

TRACE = False  # set by test.py to get a profile
_LAST_RESULTS = {}  # test.py reads exec_time_ns from here


def _build_program():
    import concourse.bacc as bacc
    import concourse.mybir as mybir
    import concourse.tile as tile

    f32 = mybir.dt.float32
    f16 = mybir.dt.float16

    nc = bacc.Bacc("TRN2", target_bir_lowering=False, debug=False)

    x_d = nc.dram_tensor("x", (IMGS_PER_CORE, CIN, HP, WP), f16,
                         kind="ExternalInput")
    w_d = nc.dram_tensor("w", (128, 2 * 9 * NDICT), f16, kind="ExternalInput")
    m_d = nc.dram_tensor("m", (NDICT, COUT), f16, kind="ExternalInput")
    out_d = nc.dram_tensor("out", (IMGS_PER_CORE, COUT, H, W), f16,
                           kind="ExternalOutput")

    with tile.TileContext(nc) as tc:
        with (
            tc.tile_pool(name="warm", bufs=1) as warm,
            tc.tile_pool(name="consts", bufs=1) as consts,
            tc.tile_pool(name="xpool", bufs=1) as xpool,
            tc.tile_pool(name="ypool", bufs=3) as ypool,
            tc.tile_pool(name="opool", bufs=2) as opool,
            tc.tile_pool(name="psum_w", bufs=1, space="PSUM") as psum_w_pool,
            tc.tile_pool(name="psum_y", bufs=2, space="PSUM") as psum_y_pool,
            tc.tile_pool(name="psum_o", bufs=4, space="PSUM") as psum_o_pool,
        ):
            # --- PE pre-warm: dummy matmuls on zeroed scratch keep the PE
            # clock ramping while the first weight/input DMAs are in flight
            scratch = warm.tile([128, 128], f16)
            nc.vector.memset(scratch[:], 0.0)
            pwarm = psum_w_pool.tile([128, 128], f32)
            for _ in range(27):
                nc.tensor.matmul(pwarm[:], scratch[:], scratch[:],
                                 start=True, stop=True)

            # --- constants + input, ordered so the first conv tile's
            # dependencies (w cb0 taps, x img0 chunk0) land first
            w_sb = consts.tile([128, 2 * 9 * NDICT], f16)
            m_sb = consts.tile([NDICT, COUT], f16)
            x_sb = xpool.tile([128, IMGS_PER_CORE, 2, HP, WP], f16, tag="x_sb")
            x_v = x_d.rearrange("i (b c) h w -> c i b h w", c=128)

            def dma_x(img, cb, chunk):
                r0, nr = ROW_CHUNKS[chunk]
                nc.sync.dma_start(x_sb[:, img, cb, r0:r0 + nr, :],
                                  x_v[:, img, cb, r0:r0 + nr, :])

            nc.sync.dma_start(w_sb[:, :900], w_d[:, :900])
            dma_x(0, 0, 0)
            nc.sync.dma_start(w_sb[:, 900:], w_d[:, 900:])
            dma_x(0, 1, 0)
            nc.sync.dma_start(m_sb[:], m_d[:])
            for img in range(IMGS_PER_CORE):
                for chunk in range(1 if img == 0 else 0, len(ROW_CHUNKS)):
                    for cb in range(2):
                        dma_x(img, cb, chunk)

            out_v = out_d.rearrange("i (b o) h w -> i b o (h w)", o=128)
            # same tensor viewed [img, o, b, hw] so a [128, 4, cols] SBUF
            # tile maps element-for-element in the tail DMA
            out_vt = out_d.rearrange("i (b o) h w -> i o b (h w)", o=128)

            def emit_conv_half(py, img, ti_idx, cb):
                h0, rows = TILES[ti_idx]
                fd = rows * W
                for ti in range(3):
                    for tj in range(3):
                        k = cb * 9 + ti * 3 + tj
                        tap = slice(k * NDICT, (k + 1) * NDICT)
                        rh = (slice(None), img, cb,
                              slice(h0 + ti, h0 + ti + rows),
                              slice(tj, tj + W))
                        nc.tensor.matmul(
                            py[:, :fd], w_sb[:, tap], x_sb[rh],
                            start=(k == 0), stop=(k == 17))

            o_accs = {}

            def emit_y_copy(py, img, ti_idx):
                _, rows = TILES[ti_idx]
                fd = rows * W
                if ti_idx == 0:
                    o_accs[img] = opool.tile([128, 4, HW], f16, tag="oacc",
                                             name=f"oacc{img}")
                y16 = ypool.tile([NDICT, MAX_FREE], f16, tag="y16")
                nc.scalar.copy(y16[:, :fd], py[:, :fd])
                return y16

            def emit_mix_half(y16, img, ti_idx, half):
                h0, rows = TILES[ti_idx]
                fd = rows * W
                c0 = h0 * W
                o_acc = o_accs[img]
                for ob in (2 * half, 2 * half + 1):
                    obs = slice(ob * 128, (ob + 1) * 128)
                    po = psum_o_pool.tile([128, MAX_FREE], f32, tag="po")
                    nc.tensor.matmul(po[:, :fd], m_sb[:, obs], y16[:, :fd],
                                     start=True, stop=True)
                    dst = o_acc[:, ob, c0:c0 + fd]
                    if ob % 2 == 0:
                        nc.vector.tensor_copy(dst, po[:, :fd])
                    else:
                        nc.scalar.copy(dst, po[:, :fd])
                # per-ob-pair DMA: output streams out as soon as each half of
                # the tile's mix lands (no end-of-kernel backlog)
                nc.sync.dma_start(
                    out_vt[img, :, 2 * half:2 * half + 2, c0:c0 + fd],
                    o_acc[:, 2 * half:2 * half + 2, c0:c0 + fd])

            # software-pipeline by one tile, with tile i-1's mix matmuls
            # interleaved into the middle of tile i's conv so its output
            # copies/DMA start ~half a tile earlier
            prev = None
            for img in range(IMGS_PER_CORE):
                for t in range(len(TILES)):
                    py = psum_y_pool.tile([NDICT, MAX_FREE], f32, tag="py",
                                          name=f"py{img}_{t}")
                    emit_conv_half(py, img, t, 0)
                    if prev is not None:
                        emit_mix_half(*prev, 0)
                    emit_conv_half(py, img, t, 1)
                    if prev is not None:
                        emit_mix_half(*prev, 1)
                    y16 = emit_y_copy(py, img, t)
                    prev = (y16, img, t)
            emit_mix_half(*prev, 0)
            emit_mix_half(*prev, 1)

    nc.compile()
    return nc


_NC_CACHE = None


def kernel(x, dictionary, lookup_indices, lookup_coefficients):
    global _NC_CACHE
    from concourse import bass_utils

    x = np.asarray(x, dtype=np.float32)
    dictionary = np.asarray(dictionary, dtype=np.float32)
    idx = np.asarray(lookup_indices).astype(np.int64)
    coef = np.asarray(lookup_coefficients, dtype=np.float32)

    # M^T[d, o] = sum_s coeff[o, s] * [idx[o, s] == d]
    mt = np.zeros((NDICT, COUT), np.float32)
    np.add.at(mt, (idx.reshape(-1),
                   np.repeat(np.arange(COUT), S)), coef.reshape(-1))

    # wt[c_in_block, (cblk, ti, tj, d)] = dictionary[d, cblk*128+c, ti, tj]
    wt = np.ascontiguousarray(
        dictionary.reshape(NDICT, 2, 128, 3, 3).transpose(2, 1, 3, 4, 0)
    ).reshape(128, 2 * 9 * NDICT)

    xp = np.pad(x, ((0, 0), (0, 0), (1, 1), (1, 1)))
    xp = np.ascontiguousarray(
        xp.reshape(N_CORES, IMGS_PER_CORE, CIN, HP, WP)).astype(np.float16)
    wt16 = wt.astype(np.float16)
    mt16 = mt.astype(np.float16)

    if _NC_CACHE is None:
        _NC_CACHE = _build_program()
    nc = _NC_CACHE

    in_maps = [{"x": xp[i], "w": wt16, "m": mt16} for i in range(N_CORES)]
    try:
        res = bass_utils.run_bass_kernel_spmd(
            nc, in_maps, core_ids=list(range(N_CORES)), trace=TRACE)
    except ModuleNotFoundError:
        # no axon NTFF profile hook in this environment
        res = bass_utils.run_bass_kernel_spmd(
            nc, in_maps, core_ids=list(range(N_CORES)), trace=False)
    _LAST_RESULTS["res"] = res

    out = np.concatenate([r["out"] for r in res.results], axis=0)
    return out.reshape(16, COUT, H, W).astype(np.float32)
